# revision 27
# baseline (speedup 1.0000x reference)
"""Trainium2 Bass kernel for relative-position attention (nn_Attention).

Reference computation (B=16, C=128, H=W=32, HEADS=4, d=32, N=1024):
    qkv  = W_qkv @ x                          (1x1 conv, per-pixel matmul)
    S    = scale * (q^T k + q^T r)            where r = rw + rh  (broadcast)
         = scale * q^T (k + r)                <- position term folds into k
    P    = softmax(S, axis=-1)
    out  = P @ v^T
Sharding: data-parallel over batch, 2 batches per core on 8 cores.

Bottleneck analysis: exp() runs on ScalarE's activation LUT (1 elem/
cycle/lane @1.2GHz; GPSIMD pow measured 100x too slow, DVE has no pow)
plus a DVE bf16-bit-trick approximation for a minority of score groups
(Schraudolph: bits(bf16 e^x) ~= int16(x*A+B), one tensor_scalar), so the
exp wall is split across two engines.  Everything else is built to keep
those two streams dense:

  - S matmuls (contraction d=32) run 4-way ROW-TILED in bf16: head h's
    [32,128] x [32,512] matmul occupies PE row strip (32h, 0); all four
    heads stream concurrently, and bf16 gets fast weight loads.
  - Scores stream through a 3-deep ring of 2-bank PSUM buffers
    ([128, 1024]); each full buffer drains with ONE wide exp op.
  - O matmuls (bf16) use the ones-column trick (M=33: 32 d rows +
    softmax-Z row), 2-way COL-TILED: the head pair at tile_position
    (0,0)/(0,64) writes PSUM partitions [0:33]/[64:97] of one bank,
    accumulating over j-chunks.  O emission is SKEWED ~2 j-chunks behind
    the scores and allowed to spill across phase boundaries, so the PE
    never executes a burst of trailing O matmuls that would stall the
    next phase's score stream (PE is in-order).
  - Attention runs in i-halves (512 query cols): psS 3x2 + psO 2x1 = 8
    PSUM banks.
  - softmax normalize: O accumulators drain to SBUF (partition-shifted
    per head), Z rows collect in a ones-initialized tile, one
    approx-reciprocal covers all 4 heads, and a block-ones PE matmul
    broadcasts 1/Z across partitions in PSUM (no DRAM bounce).  The
    normalize is emitted in two pieces a few j-chunks into the next
    phase so its PE matmul hides in the stream.
  - Two throwaway f32 matmuls at t=0 flip the PE HAM clock gate to
    full rate before the pipeline starts.
"""

import os as _os

import numpy as np

B, C, H, W = 16, 128, 32, 32
HEADS = 4
D = C // HEADS          # 32
N = H * W               # 1024
SCALE = float(D) ** -0.5
NCORES = 8
BPC = B // NCORES       # batches per core
NH = N // 2             # i-half width (512)
SLG = 2                 # 512-col slices per exp group / psS buffer
NG = 32 // SLG          # exp groups per i-half

KSCH = int(_os.environ.get("KSCH", "5"))   # exp groups per i-half on DVE
OSKEW = int(_os.environ.get("KOSKEW", "2"))  # O-matmul skew in j-chunks
# bf16-space Schraudolph exp: bits(bf16 e^x) ~= int16(x*A + B)
SCH_A = 128.0 / 0.6931471805599453
SCH_B = 128.0 * (127.0 - 0.0436)


def _build_kernel(nc, tc, tile, mybir, x_ap, wT_ap, rw_ap, rh_ap, blk1_ap,
                  out_ap):
    import concourse.bass as bass
    from concourse.masks import make_identity

    f32 = mybir.dt.float32
    f32r = mybir.dt.float32r
    bf16 = mybir.dt.bfloat16
    AF = mybir.ActivationFunctionType

    const = tc.alloc_tile_pool(name="const", bufs=1)
    sb = tc.alloc_tile_pool(name="sb", bufs=2)
    epool = tc.alloc_tile_pool(name="epool", bufs=6)
    vtpool = tc.alloc_tile_pool(name="vtpool", bufs=2)
    psS = tc.alloc_tile_pool(name="psS", bufs=3, space="PSUM")    # 3x2 banks
    psO = tc.alloc_tile_pool(name="psO", bufs=2, space="PSUM")    # 2x1 bank
    dscratch = tc.alloc_tile_pool(name="dscratch", bufs=2, space="DRAM")

    # --- constants / replicated inputs ---
    identity = const.tile([128, 128], f32)
    make_identity(nc, identity[:])
    ones_f = const.tile([128, 4], f32)
    nc.vector.memset(ones_f[:], 1.0)
    # zcomb rows {0,32,64,96} receive per-head Z; the rest stay 1.0 so the
    # whole-tile reciprocal is safe. blk1[k, m] = (k == 32*(m//32)) routes
    # row 32h of rz to output partitions [32h:32h+32] in the broadcast mm.
    zcomb = const.tile([128, NH], f32)
    nc.vector.memset(zcomb[:], 1.0)
    blk1_f = const.tile([128, 128], f32)
    nc.sync.dma_start(out=blk1_f[:], in_=blk1_ap[:])
    blk1_s = const.tile([128, 128], bf16)
    nc.vector.tensor_copy(out=blk1_s[:], in_=blk1_f[:])

    # Two throwaway f32 matmuls (~3.4us of PE busy, no DMA deps) flip the
    # HAM clock gate to 8/8 before the real pipeline starts.
    for wmm in range(2):
        ps_w = psO.tile([128, NH], f32, tag="o", name=f"warm{wmm}")
        nc.tensor.matmul(ps_w[:], lhsT=zcomb[:, 0:128], rhs=zcomb[:],
                         start=True, stop=True)

    # prefetch batch 0's x before the (smaller) weight DMAs
    x_s0 = sb.tile([128, N], f32, tag="x", name="x0_s")
    for nf in range(2):
        nc.sync.dma_start(out=x_s0[:, nf * NH:(nf + 1) * NH],
                          in_=x_ap[0, :, nf * NH:(nf + 1) * NH])
    w_s = const.tile([128, 3 * C], f32)
    nc.sync.dma_start(out=w_s[:], in_=wT_ap[:])
    rw_s = const.tile([128, W], f32)
    nc.sync.dma_start(out=rw_s[:], in_=rw_ap[:])
    rh_s = const.tile([128, H], f32)
    nc.sync.dma_start(out=rh_s[:], in_=rh_ap[:])

    w_r = const.tile([128, 3 * C], f32r)
    nc.vector.tensor_copy(out=w_r[:], in_=w_s[:])

    # r[p, y*W + x] = rw[p, x] + rh[p, y] in one op via step-0 free dims
    r_s = const.tile([128, N], f32)
    rw_b = bass.AP(tensor=rw_s.tensor, offset=rw_s.offset,
                   ap=[list(rw_s.ap[0]), [0, H], list(rw_s.ap[1])])
    rh_b = bass.AP(tensor=rh_s.tensor, offset=rh_s.offset,
                   ap=[list(rh_s.ap[0]), list(rh_s.ap[1]), [0, W]])
    nc.gpsimd.tensor_add(
        out=r_s[:].rearrange("p (y x) -> p y x", y=H), in0=rh_b, in1=rw_b
    )

    # --- x loads + rounding casts for both batches up front ---
    x_rs = []
    for b in range(BPC):
        if b == 0:
            x_s = x_s0
        else:
            x_s = sb.tile([128, N], f32, tag="x", name=f"x{b}_s")
            for nf in range(2):
                nc.sync.dma_start(out=x_s[:, nf * NH:(nf + 1) * NH],
                                  in_=x_ap[b, :, nf * NH:(nf + 1) * NH])
        x_r = sb.tile([128, N], f32r, tag="xr", name=f"x{b}_r")
        eng = nc.vector if b == 0 else nc.gpsimd
        for nf in range(2):
            sl = slice(nf * NH, (nf + 1) * NH)
            eng.tensor_copy(out=x_r[:, sl], in_=x_s[:, sl])
        x_rs.append(x_r)

    qs, kps, vs = {}, {}, {}
    vts = {0: [None] * 8, 1: [None] * 8}
    outs = {}

    def emit_qkv(b, ms):
        """Project q(m=0)/k(m=1)/v(m=2) for batch b through the psS ring."""
        for m in ms:
            ps = psS.tile([128, SLG * NH], f32, tag="sp", name=f"qkv{b}_{m}")
            for nf in range(2):
                nc.tensor.matmul(
                    ps[:, nf * NH:(nf + 1) * NH],
                    lhsT=w_r[:, m * 128:(m + 1) * 128],
                    rhs=x_rs[b][:, nf * NH:(nf + 1) * NH],
                    start=True, stop=True,
                )
            if m == 0:
                qs[b] = sb.tile([128, N], bf16, tag="q", name=f"q{b}")
                nc.scalar.copy(out=qs[b][:], in_=ps[:, 0:N])
            elif m == 1:
                kps[b] = sb.tile([128, N], bf16, tag="kp", name=f"kp{b}")
                nc.vector.tensor_add(out=kps[b][:], in0=ps[:, 0:N], in1=r_s[:])
            else:
                vs[b] = sb.tile([128, N], f32, tag="v", name=f"v{b}")
                nc.vector.tensor_copy(out=vs[b][:], in_=ps[:, 0:N])

    def emit_transposes(b, jcs):
        """v^T tiles with ones column via the psO ring."""
        for jc in jcs:
            ps_t = psO.tile([128, NH], f32, tag="o", name=f"tr{b}_{jc}")
            nc.tensor.transpose(ps_t[:, 0:128],
                                vs[b][:, jc * 128:(jc + 1) * 128], identity[:])
            vtile = vtpool.tile([128, HEADS, D + 1], bf16, tag=f"vt{jc}",
                                name=f"vt{b}_{jc}")
            nc.gpsimd.tensor_copy(
                out=vtile[:, :, D:D + 1],
                in_=ones_f[:, 0:HEADS].rearrange("p (h o) -> p h o", o=1),
            )
            nc.vector.tensor_copy(
                out=vtile[:, :, 0:D],
                in_=ps_t[:, 0:128].rearrange("p (h d) -> p h d", h=HEADS),
            )
            vts[b][jc] = vtile

    # -------- the fused phase stream --------
    sch_marks = ({int(i * NG / KSCH) + 2 for i in range(KSCH)}
                 if KSCH else set())

    class Phase:
        def __init__(self, b, ih):
            self.b, self.ih = b, ih
            self.isl = slice(ih * NH, (ih + 1) * NH)
            self.e_tiles = [None] * NG
            self.sp_cur = None
            self.oacc = [None, None]
            self.oq = []          # (jc, p) O-work not yet emitted
            self.rz = None

        def emit_s(self, jc):
            b, ih = self.b, self.ih
            jsl = slice(jc * 128, (jc + 1) * 128)
            for h in range(HEADS):
                s = 4 * jc + h
                g, off = s // SLG, s % SLG
                if off == 0:
                    self.sp_cur = psS.tile([128, SLG * NH], f32, tag="sp",
                                           name=f"s{b}_{ih}_{g}")
                nc.tensor.matmul(
                    self.sp_cur[:, off * NH:(off + 1) * NH],
                    lhsT=kps[b][32 * h:32 * h + 32, jsl],
                    rhs=qs[b][32 * h:32 * h + 32, self.isl],
                    start=True, stop=True,
                    tile_position=(32 * h, 0),
                )
                if off == SLG - 1:
                    et = epool.tile([128, SLG * NH], bf16, tag="e",
                                    name=f"e{b}_{ih}_{g}")
                    if g in sch_marks:
                        nc.vector.tensor_scalar(
                            out=et[:].bitcast(mybir.dt.int16),
                            in0=self.sp_cur[:],
                            scalar1=SCH_A, scalar2=SCH_B,
                            op0=mybir.AluOpType.mult, op1=mybir.AluOpType.add)
                    else:
                        nc.scalar.activation(out=et[:], in_=self.sp_cur[:],
                                             func=AF.Exp)
                    self.e_tiles[g] = et
            self.oq += [(jc, 0), (jc, 1)]

        def emit_o_ready(self, upto_jc):
            """Emit queued O matmuls whose E is ready, up to chunk upto_jc."""
            for t in list(self.oq):
                jc, p = t
                if jc > upto_jc:
                    continue
                if self.e_tiles[(4 * jc + 2 * p + 1) // SLG] is None:
                    continue
                self.oq.remove(t)
                if self.oacc[p] is None:
                    self.oacc[p] = psO.tile(
                        [128, NH], f32, tag="o",
                        name=f"o{self.b}_{self.ih}_{p}")
                for side, cpos in ((0, 0), (1, 64)):
                    s = 4 * jc + 2 * p + side
                    g, off = s // SLG, s % SLG
                    nc.tensor.matmul(
                        self.oacc[p][cpos:cpos + D + 1, :],
                        lhsT=vts[self.b][jc][:, 2 * p + side, :],
                        rhs=self.e_tiles[g][:, off * NH:(off + 1) * NH],
                        start=(jc == 0), stop=(jc == 7),
                        tile_position=(0, cpos),
                        skip_group_check=True,
                    )

        def norm_a(self):
            """Drain O accumulators + Z rows to SBUF, reciprocal."""
            b, ih = self.b, self.ih
            self.oaS = sb.tile([128, NH], f32, tag="oaS", name=f"oaS{b}{ih}")
            for p in range(2):
                for side, cpos in ((0, 0), (1, 64)):
                    h = 2 * p + side
                    if h % 2:
                        nc.scalar.copy(out=self.oaS[h * D:(h + 1) * D, :],
                                       in_=self.oacc[p][cpos:cpos + D, :])
                        nc.scalar.copy(
                            out=zcomb[h * D:h * D + 1, :],
                            in_=self.oacc[p][cpos + D:cpos + D + 1, :])
                    else:
                        nc.vector.tensor_copy(
                            out=self.oaS[h * D:(h + 1) * D, :],
                            in_=self.oacc[p][cpos:cpos + D, :])
                        nc.vector.tensor_copy(
                            out=zcomb[h * D:h * D + 1, :],
                            in_=self.oacc[p][cpos + D:cpos + D + 1, :])
            rzf = sb.tile([128, NH], f32, tag="rzf", name=f"rzf{b}{ih}")
            nc.vector.reciprocal_approx_fast(out=rzf[:], in_=zcomb[:])
            self.rz = sb.tile([128, NH], bf16, tag="rz", name=f"rz{b}{ih}")
            nc.gpsimd.tensor_copy(out=self.rz[:], in_=rzf[:])

        def norm_b(self):
            """Broadcast 1/Z via block-ones PE matmul, multiply, store."""
            b = self.b
            rb_ps = psO.tile([128, NH], f32, tag="o", name=f"rb{b}{self.ih}")
            nc.tensor.matmul(rb_ps[:], lhsT=blk1_s[:], rhs=self.rz[:],
                             start=True, stop=True)
            nc.vector.tensor_mul(out=outs[b][:, self.isl], in0=rb_ps[:],
                                 in1=self.oaS[:])
            nc.sync.dma_start(out=out_ap[b, :, self.isl],
                              in_=outs[b][:, self.isl])

    emit_qkv(0, (1, 0))
    outs[0] = sb.tile([128, N], f32, tag="out", name="out0")
    outs[1] = sb.tile([128, N], f32, tag="out", name="out1")

    phases = [Phase(b, ih) for b in range(BPC) for ih in range(2)]
    prev = None      # phase whose O tail / normalize is still draining
    for ph in phases:
        b, ih = ph.b, ph.ih
        for jc in range(8):
            ph.emit_s(jc)
            # deferred projection / transpose splices
            if ih == 0 and jc == 0 and b == 0:
                emit_qkv(0, (2,))
            if ih == 0 and jc == 2:
                emit_transposes(b, range(8))
            if b == 0 and ih == 1 and jc in (2, 4, 6):
                emit_qkv(1, ((1,), None, (0,), None, (2,))[jc - 2])
            # trailing O work + normalize of the previous phase
            if prev is not None:
                prev.emit_o_ready(7)
                if jc == 1 and not prev.oq:
                    prev.norm_a()
                if jc == 4:
                    if prev.rz is None:
                        prev.emit_o_ready(7)
                        prev.norm_a()
                    prev.norm_b()
                    prev = None
            # this phase's O matmuls, skewed behind the scores
            ph.emit_o_ready(jc - OSKEW)
        prev = ph

    # drain the final phase
    prev.emit_o_ready(7)
    prev.norm_a()
    prev.norm_b()

    for pool in (dscratch, psO, psS, vtpool, epool, sb, const):
        pool.release()


def build_nc():
    """Build the Bass module (shared by kernel() and test harnesses)."""
    import concourse.bacc as bacc
    import concourse.tile as tile
    from concourse import mybir

    f32 = mybir.dt.float32
    nc = bacc.Bacc("TRN2", target_bir_lowering=False, debug=False,
                   num_devices=NCORES)
    x_ap = nc.dram_tensor("x", [BPC, C, N], f32, kind="ExternalInput").ap()
    wT_ap = nc.dram_tensor("wT", [C, 3 * C], f32, kind="ExternalInput").ap()
    rw_ap = nc.dram_tensor("rw2", [HEADS * D, W], f32, kind="ExternalInput").ap()
    rh_ap = nc.dram_tensor("rh2", [HEADS * D, H], f32, kind="ExternalInput").ap()
    blk1_ap = nc.dram_tensor("blk1", [C, C], f32, kind="ExternalInput").ap()
    out_ap = nc.dram_tensor("out", [BPC, C, N], f32, kind="ExternalOutput").ap()

    with tile.TileContext(nc) as tc:
        _build_kernel(nc, tc, tile, mybir, x_ap, wT_ap, rw_ap, rh_ap, blk1_ap,
                      out_ap)
    nc.compile()
    return nc


def make_in_maps(x, W_qkv, rw, rh):
    x_ = np.ascontiguousarray(np.asarray(x, np.float32).reshape(B, C, N))
    wT = np.ascontiguousarray(np.asarray(W_qkv, np.float32).T)
    wT[:, 0:C] *= SCALE    # fold the attention score scale into q projection
    rw_ = np.ascontiguousarray(np.asarray(rw, np.float32).reshape(HEADS * D, W))
    rh_ = np.ascontiguousarray(np.asarray(rh, np.float32).reshape(HEADS * D, H))
    blk1 = np.zeros((C, C), np.float32)
    for m in range(C):
        blk1[D * (m // D), m] = 1.0
    return [
        {"x": x_[i * BPC:(i + 1) * BPC], "wT": wT, "rw2": rw_, "rh2": rh_,
         "blk1": blk1}
        for i in range(NCORES)
    ]


def kernel(x, W_qkv, rw, rh):
    from concourse.bass_utils import run_bass_kernel_spmd

    nc = build_nc()
    in_maps = make_in_maps(x, W_qkv, rw, rh)
    res = None
    for attempt in range(3):
        try:
            res = run_bass_kernel_spmd(nc, in_maps, list(range(NCORES)))
            break
        except Exception:
            # transient device errors usually clear on retry
            if attempt == 2:
                raise
    out = np.concatenate([r["out"] for r in res.results], axis=0)
    return out.reshape(B, C, H, W).astype(np.float32)
